# revision 23
# baseline (speedup 1.0000x reference)
"""Causal self-attention (B=2, T=2048, D=1024, H=16, hd=64) on 8 TRN2 cores.

Sharding: 2 batches x 4 head-groups (4 heads each). Each core computes the
full pipeline for its (batch, head-group): qkv projection (transposed
layout), causal attention, and its partial output projection. The host sums
the 4 per-batch partials (tensor-parallel reduce) and adds bproj.

Device-side layout notes:
 - x is passed pre-transposed (xT [D, T]) so the qkv projection can contract
   over D on the partition dimension.
 - Scores are computed transposed (St = k @ qT, [k_tok, q_tok]) so softmax's
   exp feeds straight into att@v as the moving operand without transposes.
 - Softmax has no max-subtraction (scores are O(6) here, exp is safe) and the
   denominator is produced by augmenting v with a ones column (M=65 matmul).
 - The 1/sqrt(hd) scale is folded into Wq/bq on the host.
 - Engine budget: ACT does exp + the rec-broadcast copy; DVE does the
   PSUM-exits (qkv bias, y-normalize, proj copy, v-transpose copies) and the
   fast reciprocal; GpSimd does diag memset + causal-mask muls (SBUF only).
 - proj contracts 128 (two heads stacked); odd heads reach partitions 64:128
   of the stacked y via a small SBUF->SBUF DMA (DVE cannot shift partitions).
"""

import sys

sys.path.insert(0, "/opt/trn_rl_repo")

import numpy as np
import ml_dtypes
from collections import deque

BF = ml_dtypes.bfloat16

B, T, D = 2, 2048, 1024
N_HEAD = 16
HD = 64  # head dim
HPC = 4  # heads per core
N_CORES = 8

P = 128
NJ = 512  # q-slice width
JT = T // NJ  # 4 q-slices
KT = D // P  # 8 contraction tiles for qkv
MT = 6  # qkv m-tiles: 2 q, 2 k, 2 v (128 dims each)
NQKV = MT * P  # 768
IT = T // P  # 16 k-token tiles

_CACHE = {}
DEBUG = False


def _build():
    import concourse.bass as bass  # noqa: F401
    import concourse.mybir as mybir
    from concourse.ap import AP
    import concourse.tile as tile
    from concourse import bacc

    F32 = mybir.dt.float32
    F32R = mybir.dt.float32r
    AF = mybir.ActivationFunctionType

    class _Bacc(bacc.Bacc):
        def insert_act_table_loads(self):
            # Exp (attention stream) and Ln (softmax denominators)
            # interleave on the ACT queue; left to itself the table-set
            # chooser picks exp_and_others + natural_log and reloads the
            # table RAMs (~1.3us) on every transition. Strip Exp/Ln from
            # every set except natural_log_exp_and_others so both resolve
            # to the one set that holds them jointly -> a single load.
            from concourse.hw_specs import get_activation_tables
            import bass_rust as _br

            AF2 = mybir.ActivationFunctionType
            has_activation = any(
                isinstance(i, mybir.InstActivation)
                for b in self.main_func.blocks
                for i in b.instructions
            )
            if not has_activation:
                return
            tables = []
            for name, fns in get_activation_tables(self.m.arch).items():
                if name != "natural_log_exp_and_others":
                    fns = fns - {AF2.Exp, AF2.Ln}
                tables.append((name, fns))
            _br.insert_act_table_loads(self, tables)

    nc = _Bacc(None, target_bir_lowering=False)
    xT_d = nc.dram_tensor("xT", [D, T], F32R, kind="ExternalInput")
    wqkv_d = nc.dram_tensor("wqkv", [D, NQKV], F32R, kind="ExternalInput")
    bqkv_d = nc.dram_tensor("bqkv2", [P, MT], F32, kind="ExternalInput")
    wproj_d = nc.dram_tensor("wproj", [P, 2 * D], F32R, kind="ExternalInput")
    masks_d = nc.dram_tensor("masks", [P, P], F32R, kind="ExternalInput")
    ident_d = nc.dram_tensor("ident", [P, P], F32R, kind="ExternalInput")
    out_d = nc.dram_tensor("out", [T, D], F32, kind="ExternalOutput")
    if DEBUG:
        dbg_qkv_d = nc.dram_tensor("dbg_qkv", [P, MT * T], F32, kind="ExternalOutput")
        dbg_vnat_d = nc.dram_tensor("dbg_vnat", [P, 2 * IT * 192], F32, kind="ExternalOutput")
        dbg_yt2_d = nc.dram_tensor("dbg_yt2", [P, 2 * T], F32, kind="ExternalOutput")
        dbg_rec_d = nc.dram_tensor("dbg_rec", [P, 8 * 1024], F32, kind="ExternalOutput")

    with tile.TileContext(nc) as tc:
        with (
            tc.tile_pool(name="const", bufs=1) as const,
            tc.tile_pool(name="xp", bufs=2) as xp,
            tc.tile_pool(name="stps", bufs=2, space="PSUM") as stps,
            tc.tile_pool(name="miscp", bufs=2, space="PSUM") as miscp,
            tc.tile_pool(name="yps", bufs=1, space="PSUM") as yps,
            tc.tile_pool(name="expp", bufs=6) as expp,
            tc.tile_pool(name="recp", bufs=2) as recp,
            tc.tile_pool(name="bcp", bufs=2) as bcp,
            tc.tile_pool(name="outp", bufs=2) as outp,
        ):
            w_sb = const.tile([P, KT, NQKV], F32R)
            bias_sb = const.tile([P, MT], F32)
            wp_sb = const.tile([P, 2, D], F32R)
            masks_sb = const.tile([P, P], F32R)
            ident = const.tile([P, P], F32R)
            qkvT_sb = const.tile([P, MT, T], F32R)
            vnat_sb = const.tile([P, 2, IT, 192], F32R)
            yt2_sb = const.tile([P, 2, T], F32R)

            w_r = wqkv_d.rearrange("(kt p) n -> p kt n", p=P)
            for k in range(KT):
                nc.sync.dma_start(w_sb[:, k, :], w_r[:, k, :])
            nc.sync.dma_start(bias_sb[:], bqkv_d[:])
            nc.sync.dma_start(
                wp_sb[:], wproj_d.rearrange("p (g d) -> p g d", g=2)
            )
            nc.sync.dma_start(masks_sb[:], masks_d[:])
            nc.sync.dma_start(ident[:], ident_d[:])

            xT_r = xT_d.rearrange("(kt p) t -> p kt t", p=P)

            # ---- Stage 1+2 as schedulable units ------------------------
            # qkv projection groups and v-transposes for q-slice j+1 are
            # interleaved into attention slice j's loop as dense, wait-free
            # PE filler (keeps the PE activity monitor warm).
            xts = {}

            def emit_xt(j):
                xt = xp.tile([P, KT, NJ], F32R, tag="xt", name=f"xt{j}")
                for k in range(KT):
                    nc.sync.dma_start(
                        xt[:, k, :], xT_r[:, k, j * NJ : (j + 1) * NJ]
                    )
                xts[j] = xt

            qkv_ps = {}

            def emit_qkv_pair(j, m, pair):
                if pair == 0:
                    qkv_ps[(j, m)] = miscp.tile(
                        [P, NJ], F32, tag="misc", name=f"qkvps{j}_{m}"
                    )
                ps = qkv_ps[(j, m)]
                for k in range(2 * pair, 2 * pair + 2):
                    nc.tensor.matmul(
                        ps[:],
                        w_sb[:, k, m * P : (m + 1) * P],
                        xts[j][:, k, :],
                        start=(k == 0),
                        stop=(k == KT - 1),
                    )
                if pair == 3:
                    del qkv_ps[(j, m)]
                    nc.vector.tensor_scalar_add(
                        qkvT_sb[:, m, j * NJ : (j + 1) * NJ],
                        ps[:],
                        bias_sb[:, m : m + 1],
                    )

            def emit_vt(h2, ii):
                pt = miscp.tile([P, NJ], F32R, tag="misc", name=f"vt{h2}_{ii}")
                nc.tensor.transpose(
                    pt[:, 0:P], qkvT_sb[:, 4 + h2, ii * P : (ii + 1) * P], ident[:]
                )
                nc.vector.tensor_copy(vnat_sb[:, h2, ii, 0:HD], pt[:, 0:HD])
                nc.vector.tensor_copy(
                    vnat_sb[:, h2, ii, 2 * HD : 2 * HD + P - HD], pt[:, HD:P]
                )

            fill_q = deque()

            def push_fill(j):
                for m in range(MT):
                    for pair in range(4):
                        fill_q.append(("qkv", j, m, pair))
                for h2 in range(2):
                    for ii in range(4 * j, 4 * j + 4):
                        fill_q.append(("vt", j, h2, ii))

            def emit_fill(item):
                if item[0] == "qkv":
                    emit_qkv_pair(item[1], item[2], item[3])
                else:
                    emit_vt(item[2], item[3])

            for h2 in range(2):
                nc.gpsimd.memset(vnat_sb[:, h2, :, :].bitcast(F32), 1.0)
            emit_xt(0)
            push_fill(0)
            while fill_q:
                emit_fill(fill_q.popleft())

            # ---- Stage 3: attention per head-pair ----------------------
            # Software-pipelined: St(i) is issued before Y(i-1) so ACT's
            # exp(i-1) overlaps the PE's St(i); normalization of slice (hp,j)
            # is deferred into slice (hp,j)+1's loop so the reciprocal's
            # latency hides behind matmul work.

            def emit_recip(pend):
                # 1/d = exp(-ln d) on ACT (ln+exp share one table set). The
                # Y matmuls deliver d pre-broadcast: par0's on partitions
                # 64:128, par1's on 0:64. A small SBUF->SBUF DMA moves each
                # reciprocal to the partition range its y values occupy
                # (engines cannot cross partitions; DMA can).
                hp_, j_, y2_ = pend
                tln = recp.tile([P, 2, NJ], F32, tag="rec")
                nc.scalar.activation(
                    tln[64:128, 0, :], y2_[64:128, 0, :], AF.Ln
                )
                nc.scalar.activation(
                    tln[0:HD, 1, :], y2_[0:HD, 1, :], AF.Ln
                )
                recb = recp.tile([P, 2, NJ], F32, tag="recb")
                nc.scalar.activation(
                    recb[64:128, 0, :], tln[64:128, 0, :], AF.Exp, scale=-1.0
                )
                nc.scalar.activation(
                    recb[0:HD, 1, :], tln[0:HD, 1, :], AF.Exp, scale=-1.0
                )
                rec2 = bcp.tile([P, 2, NJ], F32, tag="bc")
                nc.sync.dma_start(rec2[0:HD, 0, :], recb[64:128, 0, :])
                nc.sync.dma_start(rec2[64:128, 1, :], recb[0:HD, 1, :])
                if DEBUG:
                    nc.sync.dma_start(
                        dbg_rec_d.rearrange("p (s x) -> p s x", s=8)[
                            :, 2 * j_ + hp_, 0:1024
                        ],
                        rec2[:, :, :].rearrange("p a b -> p (a b)"),
                    )
                return (hp_, j_, y2_, rec2)

            def emit_norm(pend):
                hp_, j_, y2_, rec2 = pend
                # head 2*hp_ -> partitions 0:64; head 2*hp_+1 -> 64:128
                nc.vector.tensor_mul(
                    yt2_sb[0:HD, hp_, j_ * NJ : (j_ + 1) * NJ],
                    y2_[0:HD, 0, :],
                    rec2[0:HD, 0, :],
                )
                nc.vector.tensor_mul(
                    yt2_sb[HD:P, hp_, j_ * NJ : (j_ + 1) * NJ],
                    y2_[HD:P, 1, :],
                    rec2[HD:P, 1, :],
                )

            def emit_y(rec):
                hp_, pi, pexp, y2_, last, j_ = rec
                r_ = pi - 4 * j_
                cy = P * r_ if r_ in (1, 2) else 0
                # weight layout per k-tile: [vA(0:64) | ones(64:128) | vB
                # (128:192)]. par0 uses cols 0:128 -> yA on partitions 0:64,
                # denominator replicated on 64:128. par1 uses cols 64:192 ->
                # denominator on 0:64, yB on 64:128 (where the stacked proj
                # layout wants it -- no partition shift needed).
                nc.tensor.matmul(
                    y2_[:, 0, cy:NJ],
                    vnat_sb[:, hp_, pi, 0:P],
                    pexp[:, 0, cy:NJ],
                    start=(pi == 0),
                    stop=last,
                )
                nc.tensor.matmul(
                    y2_[:, 1, cy:NJ],
                    vnat_sb[:, hp_, pi, HD : HD + P],
                    pexp[:, 1, cy:NJ],
                    start=(pi == 0),
                    stop=last,
                )

            def emit_proj(item):
                qm_i, n = item
                po = miscp.tile([P, NJ], F32, tag="misc")
                for g in range(2):
                    nc.tensor.matmul(
                        po[:, :],
                        yt2_sb[:, g, qm_i * P : (qm_i + 1) * P],
                        wp_sb[:, g, n * NJ : (n + 1) * NJ],
                        start=(g == 0),
                        stop=(g == 1),
                    )
                ot = outp.tile([P, NJ], F32)
                nc.vector.tensor_copy(ot[:], po[:, :])
                nc.sync.dma_start(
                    out_d[qm_i * P : (qm_i + 1) * P, n * NJ : (n + 1) * NJ], ot[:]
                )

            # Global software pipeline over slices (j outer, hp inner):
            # y matmuls trail the St/exp stream by DEPTH iterations and spill
            # across slice boundaries; each slice's normalization runs inside
            # a later slice's loop; proj groups for q-slice j are spread one
            # per iteration once both head-pairs of j are normalized.
            DEPTH = 4
            y_q = deque()  # (hp, i, exp2, y2holder, last, j)
            norm_q = deque()
            proj_items = deque()  # (qm_i, n)
            y2_map = {}  # (hp, j) -> y2 tile, allocated lazily at first y

            def emit_y2(rec):
                hp_, pi, pexp, _, last, j_ = rec
                if pi == 0:
                    y2_map[(hp_, j_)] = yps.tile(
                        [P, 2, NJ], F32, tag="y", name=f"y2_{hp_}_{j_}"
                    )
                emit_y((hp_, pi, pexp, y2_map[(hp_, j_)], last, j_))
                if last:
                    # defer the norm chain by one unit so its Ln does not
                    # head-of-line-block ready exps in the ACT FIFO
                    norm_q.append((hp_, j_, y2_map.pop((hp_, j_))))

            def drain_norm():
                # called at unit START: one norm chain (ACT/DVE/DMA only),
                # emitted before this unit's y2 alloc can recycle its buffer
                if norm_q:
                    pend = norm_q.popleft()
                    emit_norm(emit_recip(pend))
                    if pend[0] == 1:
                        for qq in range(4 * pend[1], 4 * pend[1] + 4):
                            proj_items.append((qq, 0))
                            proj_items.append((qq, 1))

            def pump(i):
                # one PE fill plus maybe a proj per unit: fills keep the PE
                # dense while ACT chews exp
                if fill_q:
                    emit_fill(fill_q.popleft())
                if proj_items:
                    emit_proj(proj_items.popleft())
                elif fill_q and len(fill_q) > 12:
                    emit_fill(fill_q.popleft())

            for j in range(JT):
                if j + 1 < JT:
                    emit_xt(j + 1)
                    push_fill(j + 1)
                n_i = 4 * j + 4
                for hp in range(2):
                    for i in range(n_i):
                        drain_norm()
                        if len(y_q) > DEPTH:
                            emit_y2(y_q.popleft())
                        qm, km = hp, 2 + hp
                        st2 = stps.tile([P, 2, NJ], F32, tag="st")
                        r = i - 4 * j
                        c0 = P * r if r > 0 else 0
                        nc.tensor.matmul(
                            st2[:, 0, c0:NJ],
                            qkvT_sb[0:HD, km, i * P : (i + 1) * P],
                            qkvT_sb[0:HD, qm, j * NJ + c0 : (j + 1) * NJ],
                            start=True,
                            stop=True,
                            tile_position=(0, 0),
                        )
                        nc.tensor.matmul(
                            st2[:, 1, c0:NJ],
                            qkvT_sb[HD:P, km, i * P : (i + 1) * P],
                            qkvT_sb[HD:P, qm, j * NJ + c0 : (j + 1) * NJ],
                            start=True,
                            stop=True,
                            tile_position=(64, 0),
                        )
                        exp2 = expp.tile([P, 2, NJ], F32R, tag="exp")
                        if r < 0:
                            nc.scalar.activation(exp2[:], st2[:], AF.Exp)
                        else:
                            # diag block: cols [0, 128r) are fully above the
                            # causal line -> zero; cols [128r, 128r+128) are
                            # triangular; the rest is fully kept.
                            if c0 > 0:
                                nc.gpsimd.memset(exp2[:, :, 0:c0].bitcast(F32), 0.0)
                            nc.scalar.activation(
                                exp2[:, :, c0:NJ], st2[:, :, c0:NJ], AF.Exp
                            )
                            m_ap = masks_sb[:, 0:P]
                            m_bc = AP(
                                m_ap.tensor,
                                m_ap.offset,
                                [list(m_ap.ap[0]), [0, 2], list(m_ap.ap[1])],
                            )
                            nc.gpsimd.tensor_mul(
                                exp2[:, :, c0 : c0 + P],
                                exp2[:, :, c0 : c0 + P],
                                m_bc,
                            )
                        y_q.append((hp, i, exp2, None, i == n_i - 1, j))
                        pump(i)
                # slice j+1's qkv/vT must be complete before its St reads
                while fill_q:
                    emit_fill(fill_q.popleft())

            while y_q:
                emit_y2(y_q.popleft())
            drain_norm()
            drain_norm()
            while proj_items:
                emit_proj(proj_items.popleft())
            if DEBUG:
                nc.sync.dma_start(
                    dbg_qkv_d[:], qkvT_sb[:].bitcast(F32).rearrange("p a b -> p (a b)")
                )
                nc.sync.dma_start(
                    dbg_vnat_d[:],
                    vnat_sb[:].bitcast(F32).rearrange("p a b c -> p (a b c)"),
                )
                nc.sync.dma_start(
                    dbg_yt2_d[:], yt2_sb[:].bitcast(F32).rearrange("p a b -> p (a b)")
                )

    nc.compile()
    return nc


def _prep_inputs(x, Wqkv, bqkv, Wproj):
    """Per-core input maps. Core c -> batch c//4, heads 4*(c%4) .. +4."""
    scale = np.float32(1.0 / np.sqrt(HD))
    pp = np.arange(P)[:, None]
    ff = np.arange(P)[None, :]
    masks = (ff >= pp).astype(np.float32)

    in_maps = []
    for c in range(N_CORES):
        b, g = divmod(c, HPC)
        cs = slice(256 * g, 256 * g + 256)
        wq = Wqkv[:, 0 * D :][:, cs] * scale
        wk = Wqkv[:, 1 * D : 2 * D][:, cs]
        wv = Wqkv[:, 2 * D : 3 * D][:, cs]
        wqkv_c = np.ascontiguousarray(np.concatenate([wq, wk, wv], axis=1), np.float32)
        bq = bqkv[0 * D :][cs] * scale
        bk = bqkv[1 * D : 2 * D][cs]
        bv = bqkv[2 * D : 3 * D][cs]
        bqkv_c = np.concatenate([bq, bk, bv]).reshape(MT, P).T
        # head-pairs stacked on partitions: row p, pair g2, col d ->
        # Wproj[256*g + 128*g2 + p, d]
        wproj_c = (
            Wproj[256 * g : 256 * (g + 1), :]
            .reshape(2, P, D)
            .transpose(1, 0, 2)
            .reshape(P, 2 * D)
        )
        in_maps.append(
            {
                "xT": np.ascontiguousarray(x[b].T, np.float32),
                "wqkv": wqkv_c,
                "bqkv2": np.ascontiguousarray(bqkv_c, np.float32),
                "wproj": np.ascontiguousarray(wproj_c, np.float32),
                "masks": masks,
                "ident": np.eye(P, dtype=np.float32),
            }
        )
    return in_maps


def kernel(x, Wqkv, bqkv, Wproj, bproj, _trace=False, _trace_out=None):
    from concourse.bass_utils import run_bass_kernel_spmd

    if "nc" not in _CACHE:
        _CACHE["nc"] = _build()
    nc = _CACHE["nc"]

    x = np.asarray(x, np.float32)
    Wqkv = np.asarray(Wqkv, np.float32)
    bqkv = np.asarray(bqkv, np.float32)
    Wproj = np.asarray(Wproj, np.float32)
    bproj = np.asarray(bproj, np.float32)

    in_maps = _prep_inputs(x, Wqkv, bqkv, Wproj)
    res = run_bass_kernel_spmd(
        nc, in_maps, core_ids=list(range(N_CORES)), trace=_trace
    )
    if _trace_out is not None:
        _trace_out.append(res)

    out = np.empty((B, T, D), np.float32)
    for b in range(B):
        acc = res.results[HPC * b]["out"].astype(np.float32)
        for g in range(1, HPC):
            acc = acc + res.results[HPC * b + g]["out"]
        out[b] = acc + bproj[None, :]
    return out


# revision 24
# speedup vs baseline: 1.0078x; 1.0078x over previous
"""Causal self-attention (B=2, T=2048, D=1024, H=16, hd=64) on 8 TRN2 cores.

Sharding: 2 batches x 4 head-groups (4 heads each). Each core computes the
full pipeline for its (batch, head-group): qkv projection (transposed
layout), causal attention, and its partial output projection. The host sums
the 4 per-batch partials (tensor-parallel reduce) and adds bproj.

Device-side layout notes:
 - x is passed pre-transposed (xT [D, T]) so the qkv projection can contract
   over D on the partition dimension.
 - Scores are computed transposed (St = k @ qT, [k_tok, q_tok]) so softmax's
   exp feeds straight into att@v as the moving operand without transposes.
 - Softmax has no max-subtraction (scores are O(6) here, exp is safe) and the
   denominator is produced by augmenting v with a ones column (M=65 matmul).
 - The 1/sqrt(hd) scale is folded into Wq/bq on the host.
 - Engine budget: ACT does exp + the rec-broadcast copy; DVE does the
   PSUM-exits (qkv bias, y-normalize, proj copy, v-transpose copies) and the
   fast reciprocal; GpSimd does diag memset + causal-mask muls (SBUF only).
 - proj contracts 128 (two heads stacked); odd heads reach partitions 64:128
   of the stacked y via a small SBUF->SBUF DMA (DVE cannot shift partitions).
"""

import sys

sys.path.insert(0, "/opt/trn_rl_repo")

import numpy as np
import ml_dtypes
from collections import deque

BF = ml_dtypes.bfloat16

B, T, D = 2, 2048, 1024
N_HEAD = 16
HD = 64  # head dim
HPC = 4  # heads per core
N_CORES = 8

P = 128
NJ = 512  # q-slice width
JT = T // NJ  # 4 q-slices
KT = D // P  # 8 contraction tiles for qkv
MT = 6  # qkv m-tiles: 2 q, 2 k, 2 v (128 dims each)
NQKV = MT * P  # 768
IT = T // P  # 16 k-token tiles

_CACHE = {}
DEBUG = False


def _build():
    import concourse.bass as bass  # noqa: F401
    import concourse.mybir as mybir
    from concourse.ap import AP
    import concourse.tile as tile
    from concourse import bacc

    F32 = mybir.dt.float32
    F32R = mybir.dt.float32r
    AF = mybir.ActivationFunctionType

    class _Bacc(bacc.Bacc):
        def insert_act_table_loads(self):
            # Exp (attention stream) and Ln (softmax denominators)
            # interleave on the ACT queue; left to itself the table-set
            # chooser picks exp_and_others + natural_log and reloads the
            # table RAMs (~1.3us) on every transition. Strip Exp/Ln from
            # every set except natural_log_exp_and_others so both resolve
            # to the one set that holds them jointly -> a single load.
            from concourse.hw_specs import get_activation_tables
            import bass_rust as _br

            AF2 = mybir.ActivationFunctionType
            has_activation = any(
                isinstance(i, mybir.InstActivation)
                for b in self.main_func.blocks
                for i in b.instructions
            )
            if not has_activation:
                return
            tables = []
            for name, fns in get_activation_tables(self.m.arch).items():
                if name != "natural_log_exp_and_others":
                    fns = fns - {AF2.Exp, AF2.Ln}
                tables.append((name, fns))
            _br.insert_act_table_loads(self, tables)

    nc = _Bacc(None, target_bir_lowering=False)
    xT_d = nc.dram_tensor("xT", [D, T], F32R, kind="ExternalInput")
    wqkv_d = nc.dram_tensor("wqkv", [D, NQKV], F32R, kind="ExternalInput")
    bqkv_d = nc.dram_tensor("bqkv2", [P, MT], F32, kind="ExternalInput")
    wproj_d = nc.dram_tensor("wproj", [P, 2 * D], F32R, kind="ExternalInput")
    masks_d = nc.dram_tensor("masks", [P, P], F32R, kind="ExternalInput")
    ident_d = nc.dram_tensor("ident", [P, P], F32R, kind="ExternalInput")
    out_d = nc.dram_tensor("out", [T, D], F32, kind="ExternalOutput")
    if DEBUG:
        dbg_qkv_d = nc.dram_tensor("dbg_qkv", [P, MT * T], F32, kind="ExternalOutput")
        dbg_vnat_d = nc.dram_tensor("dbg_vnat", [P, 2 * IT * 192], F32, kind="ExternalOutput")
        dbg_yt2_d = nc.dram_tensor("dbg_yt2", [P, 2 * T], F32, kind="ExternalOutput")
        dbg_rec_d = nc.dram_tensor("dbg_rec", [P, 8 * 1024], F32, kind="ExternalOutput")

    with tile.TileContext(nc) as tc:
        with (
            tc.tile_pool(name="const", bufs=1) as const,
            tc.tile_pool(name="xp", bufs=2) as xp,
            tc.tile_pool(name="stps", bufs=2, space="PSUM") as stps,
            tc.tile_pool(name="miscp", bufs=2, space="PSUM") as miscp,
            tc.tile_pool(name="yps", bufs=1, space="PSUM") as yps,
            tc.tile_pool(name="expp", bufs=6) as expp,
            tc.tile_pool(name="recp", bufs=2) as recp,
            tc.tile_pool(name="bcp", bufs=2) as bcp,
            tc.tile_pool(name="outp", bufs=2) as outp,
        ):
            w_sb = const.tile([P, KT, NQKV], F32R)
            bias_sb = const.tile([P, MT], F32)
            wp_sb = const.tile([P, 2, D], F32R)
            masks_sb = const.tile([P, P], F32R)
            ident = const.tile([P, P], F32R)
            qkvT_sb = const.tile([P, MT, T], F32R)
            vnat_sb = const.tile([P, 2, IT, 192], F32R)
            yt2_sb = const.tile([P, 2, T], F32R)

            w_r = wqkv_d.rearrange("(kt p) n -> p kt n", p=P)
            for k in range(KT):
                nc.sync.dma_start(w_sb[:, k, :], w_r[:, k, :])
            nc.sync.dma_start(bias_sb[:], bqkv_d[:])
            nc.sync.dma_start(
                wp_sb[:], wproj_d.rearrange("p (g d) -> p g d", g=2)
            )
            nc.sync.dma_start(masks_sb[:], masks_d[:])
            nc.sync.dma_start(ident[:], ident_d[:])

            xT_r = xT_d.rearrange("(kt p) t -> p kt t", p=P)

            # ---- Stage 1+2 as schedulable units ------------------------
            # qkv projection groups and v-transposes for q-slice j+1 are
            # interleaved into attention slice j's loop as dense, wait-free
            # PE filler (keeps the PE activity monitor warm).
            xts = {}

            def emit_xt(j):
                xt = xp.tile([P, KT, NJ], F32R, tag="xt", name=f"xt{j}")
                for k in range(KT):
                    nc.sync.dma_start(
                        xt[:, k, :], xT_r[:, k, j * NJ : (j + 1) * NJ]
                    )
                xts[j] = xt

            qkv_ps = {}

            def emit_qkv_pair(j, m, pair):
                if pair == 0:
                    qkv_ps[(j, m)] = miscp.tile(
                        [P, NJ], F32, tag="misc", name=f"qkvps{j}_{m}"
                    )
                ps = qkv_ps[(j, m)]
                for k in range(2 * pair, 2 * pair + 2):
                    nc.tensor.matmul(
                        ps[:],
                        w_sb[:, k, m * P : (m + 1) * P],
                        xts[j][:, k, :],
                        start=(k == 0),
                        stop=(k == KT - 1),
                    )
                if pair == 3:
                    del qkv_ps[(j, m)]
                    nc.vector.tensor_scalar_add(
                        qkvT_sb[:, m, j * NJ : (j + 1) * NJ],
                        ps[:],
                        bias_sb[:, m : m + 1],
                    )

            def emit_vt(h2, ii):
                pt = miscp.tile([P, NJ], F32R, tag="misc", name=f"vt{h2}_{ii}")
                nc.tensor.transpose(
                    pt[:, 0:P], qkvT_sb[:, 4 + h2, ii * P : (ii + 1) * P], ident[:]
                )
                nc.vector.tensor_copy(vnat_sb[:, h2, ii, 0:HD], pt[:, 0:HD])
                nc.vector.tensor_copy(
                    vnat_sb[:, h2, ii, 2 * HD : 2 * HD + P - HD], pt[:, HD:P]
                )

            fill_q = deque()

            def push_fill(j):
                for m in range(MT):
                    for pair in range(4):
                        fill_q.append(("qkv", j, m, pair))
                for h2 in range(2):
                    for ii in range(4 * j, 4 * j + 4):
                        fill_q.append(("vt", j, h2, ii))

            def emit_fill(item):
                if item[0] == "qkv":
                    emit_qkv_pair(item[1], item[2], item[3])
                else:
                    emit_vt(item[2], item[3])

            for h2 in range(2):
                nc.gpsimd.memset(vnat_sb[:, h2, :, :].bitcast(F32), 1.0)
            emit_xt(0)
            push_fill(0)
            while fill_q:
                emit_fill(fill_q.popleft())

            # ---- Stage 3: attention per head-pair ----------------------
            # Software-pipelined: St(i) is issued before Y(i-1) so ACT's
            # exp(i-1) overlaps the PE's St(i); normalization of slice (hp,j)
            # is deferred into slice (hp,j)+1's loop so the reciprocal's
            # latency hides behind matmul work.

            def emit_recip(pend):
                # 1/d = exp(-ln d) on ACT (ln+exp share one table set). The
                # Y matmuls deliver d pre-broadcast: par0's on partitions
                # 64:128, par1's on 0:64. A small SBUF->SBUF DMA moves each
                # reciprocal to the partition range its y values occupy
                # (engines cannot cross partitions; DMA can).
                hp_, j_, y2_ = pend
                tln = recp.tile([P, 2, NJ], F32, tag="rec")
                nc.scalar.activation(
                    tln[64:128, 0, :], y2_[64:128, 0, :], AF.Ln
                )
                nc.scalar.activation(
                    tln[0:HD, 1, :], y2_[0:HD, 1, :], AF.Ln
                )
                recb = recp.tile([P, 2, NJ], F32, tag="recb")
                nc.scalar.activation(
                    recb[64:128, 0, :], tln[64:128, 0, :], AF.Exp, scale=-1.0
                )
                nc.scalar.activation(
                    recb[0:HD, 1, :], tln[0:HD, 1, :], AF.Exp, scale=-1.0
                )
                rec2 = bcp.tile([P, 2, NJ], F32, tag="bc")
                nc.sync.dma_start(rec2[0:HD, 0, :], recb[64:128, 0, :])
                nc.sync.dma_start(rec2[64:128, 1, :], recb[0:HD, 1, :])
                if DEBUG:
                    nc.sync.dma_start(
                        dbg_rec_d.rearrange("p (s x) -> p s x", s=8)[
                            :, 2 * j_ + hp_, 0:1024
                        ],
                        rec2[:, :, :].rearrange("p a b -> p (a b)"),
                    )
                return (hp_, j_, y2_, rec2)

            def emit_norm(pend):
                hp_, j_, y2_, rec2 = pend
                # head 2*hp_ -> partitions 0:64; head 2*hp_+1 -> 64:128
                nc.vector.tensor_mul(
                    yt2_sb[0:HD, hp_, j_ * NJ : (j_ + 1) * NJ],
                    y2_[0:HD, 0, :],
                    rec2[0:HD, 0, :],
                )
                nc.vector.tensor_mul(
                    yt2_sb[HD:P, hp_, j_ * NJ : (j_ + 1) * NJ],
                    y2_[HD:P, 1, :],
                    rec2[HD:P, 1, :],
                )

            def emit_y(rec):
                hp_, pi, pexp, y2_, last, j_ = rec
                r_ = pi - 4 * j_
                cy = P * r_ if r_ in (1, 2) else 0
                # weight layout per k-tile: [vA(0:64) | ones(64:128) | vB
                # (128:192)]. par0 uses cols 0:128 -> yA on partitions 0:64,
                # denominator replicated on 64:128. par1 uses cols 64:192 ->
                # denominator on 0:64, yB on 64:128 (where the stacked proj
                # layout wants it -- no partition shift needed).
                nc.tensor.matmul(
                    y2_[:, 0, cy:NJ],
                    vnat_sb[:, hp_, pi, 0:P],
                    pexp[:, 0, cy:NJ],
                    start=(pi == 0),
                    stop=last,
                )
                nc.tensor.matmul(
                    y2_[:, 1, cy:NJ],
                    vnat_sb[:, hp_, pi, HD : HD + P],
                    pexp[:, 1, cy:NJ],
                    start=(pi == 0),
                    stop=last,
                )

            def emit_proj(item):
                qm_i, n = item
                po = miscp.tile([P, NJ], F32, tag="misc")
                for g in range(2):
                    nc.tensor.matmul(
                        po[:, :],
                        yt2_sb[:, g, qm_i * P : (qm_i + 1) * P],
                        wp_sb[:, g, n * NJ : (n + 1) * NJ],
                        start=(g == 0),
                        stop=(g == 1),
                    )
                ot = outp.tile([P, NJ], F32)
                nc.vector.tensor_copy(ot[:], po[:, :])
                nc.sync.dma_start(
                    out_d[qm_i * P : (qm_i + 1) * P, n * NJ : (n + 1) * NJ], ot[:]
                )

            # Global software pipeline over slices (j outer, hp inner):
            # y matmuls trail the St/exp stream by DEPTH iterations and spill
            # across slice boundaries; each slice's normalization runs inside
            # a later slice's loop; proj groups for q-slice j are spread one
            # per iteration once both head-pairs of j are normalized.
            DEPTH = 4
            y_q = deque()  # (hp, i, exp2, y2holder, last, j)
            proj_items = deque()  # (qm_i, n)
            y2_map = {}  # (hp, j) -> y2 tile, allocated lazily at first y

            def emit_y2(rec):
                hp_, pi, pexp, _, last, j_ = rec
                if pi == 0:
                    y2_map[(hp_, j_)] = yps.tile(
                        [P, 2, NJ], F32, tag="y", name=f"y2_{hp_}_{j_}"
                    )
                emit_y((hp_, pi, pexp, y2_map[(hp_, j_)], last, j_))
                if last:
                    # norm chain is ACT/DVE/DMA-only: emit eagerly (no PE
                    # cost) so the single y2 PSUM buffer frees promptly
                    emit_norm(emit_recip((hp_, j_, y2_map.pop((hp_, j_)))))
                    if hp_ == 1:
                        for qq in range(4 * j_, 4 * j_ + 4):
                            proj_items.append((qq, 0))
                            proj_items.append((qq, 1))

            def pump(i):
                # one PE fill plus maybe a proj per unit: fills keep the PE
                # dense while ACT chews exp
                if fill_q:
                    emit_fill(fill_q.popleft())
                if proj_items:
                    emit_proj(proj_items.popleft())
                elif fill_q and len(fill_q) > 12:
                    emit_fill(fill_q.popleft())

            for j in range(JT):
                if j + 1 < JT:
                    emit_xt(j + 1)
                    push_fill(j + 1)
                n_i = 4 * j + 4
                for hp in range(2):
                    for i in range(n_i):
                        if len(y_q) > DEPTH:
                            emit_y2(y_q.popleft())
                        qm, km = hp, 2 + hp
                        st2 = stps.tile([P, 2, NJ], F32, tag="st")
                        r = i - 4 * j
                        c0 = P * r if r > 0 else 0
                        nc.tensor.matmul(
                            st2[:, 0, c0:NJ],
                            qkvT_sb[0:HD, km, i * P : (i + 1) * P],
                            qkvT_sb[0:HD, qm, j * NJ + c0 : (j + 1) * NJ],
                            start=True,
                            stop=True,
                            tile_position=(0, 0),
                        )
                        nc.tensor.matmul(
                            st2[:, 1, c0:NJ],
                            qkvT_sb[HD:P, km, i * P : (i + 1) * P],
                            qkvT_sb[HD:P, qm, j * NJ + c0 : (j + 1) * NJ],
                            start=True,
                            stop=True,
                            tile_position=(64, 0),
                        )
                        exp2 = expp.tile([P, 2, NJ], F32R, tag="exp")
                        if r < 0:
                            nc.scalar.activation(exp2[:], st2[:], AF.Exp)
                        else:
                            # diag block: cols [0, 128r) are fully above the
                            # causal line -> zero; cols [128r, 128r+128) are
                            # triangular; the rest is fully kept.
                            if c0 > 0:
                                nc.gpsimd.memset(exp2[:, :, 0:c0].bitcast(F32), 0.0)
                            nc.scalar.activation(
                                exp2[:, :, c0:NJ], st2[:, :, c0:NJ], AF.Exp
                            )
                            m_ap = masks_sb[:, 0:P]
                            m_bc = AP(
                                m_ap.tensor,
                                m_ap.offset,
                                [list(m_ap.ap[0]), [0, 2], list(m_ap.ap[1])],
                            )
                            nc.gpsimd.tensor_mul(
                                exp2[:, :, c0 : c0 + P],
                                exp2[:, :, c0 : c0 + P],
                                m_bc,
                            )
                        y_q.append((hp, i, exp2, None, i == n_i - 1, j))
                        pump(i)
                # slice j+1's qkv/vT must be complete before its St reads
                while fill_q:
                    emit_fill(fill_q.popleft())

            while y_q:
                emit_y2(y_q.popleft())
            while proj_items:
                emit_proj(proj_items.popleft())
            if DEBUG:
                nc.sync.dma_start(
                    dbg_qkv_d[:], qkvT_sb[:].bitcast(F32).rearrange("p a b -> p (a b)")
                )
                nc.sync.dma_start(
                    dbg_vnat_d[:],
                    vnat_sb[:].bitcast(F32).rearrange("p a b c -> p (a b c)"),
                )
                nc.sync.dma_start(
                    dbg_yt2_d[:], yt2_sb[:].bitcast(F32).rearrange("p a b -> p (a b)")
                )

    nc.compile()
    return nc


def _prep_inputs(x, Wqkv, bqkv, Wproj):
    """Per-core input maps. Core c -> batch c//4, heads 4*(c%4) .. +4."""
    scale = np.float32(1.0 / np.sqrt(HD))
    pp = np.arange(P)[:, None]
    ff = np.arange(P)[None, :]
    masks = (ff >= pp).astype(np.float32)

    in_maps = []
    for c in range(N_CORES):
        b, g = divmod(c, HPC)
        cs = slice(256 * g, 256 * g + 256)
        wq = Wqkv[:, 0 * D :][:, cs] * scale
        wk = Wqkv[:, 1 * D : 2 * D][:, cs]
        wv = Wqkv[:, 2 * D : 3 * D][:, cs]
        wqkv_c = np.ascontiguousarray(np.concatenate([wq, wk, wv], axis=1), np.float32)
        bq = bqkv[0 * D :][cs] * scale
        bk = bqkv[1 * D : 2 * D][cs]
        bv = bqkv[2 * D : 3 * D][cs]
        bqkv_c = np.concatenate([bq, bk, bv]).reshape(MT, P).T
        # head-pairs stacked on partitions: row p, pair g2, col d ->
        # Wproj[256*g + 128*g2 + p, d]
        wproj_c = (
            Wproj[256 * g : 256 * (g + 1), :]
            .reshape(2, P, D)
            .transpose(1, 0, 2)
            .reshape(P, 2 * D)
        )
        in_maps.append(
            {
                "xT": np.ascontiguousarray(x[b].T, np.float32),
                "wqkv": wqkv_c,
                "bqkv2": np.ascontiguousarray(bqkv_c, np.float32),
                "wproj": np.ascontiguousarray(wproj_c, np.float32),
                "masks": masks,
                "ident": np.eye(P, dtype=np.float32),
            }
        )
    return in_maps


def kernel(x, Wqkv, bqkv, Wproj, bproj, _trace=False, _trace_out=None):
    from concourse.bass_utils import run_bass_kernel_spmd

    if "nc" not in _CACHE:
        _CACHE["nc"] = _build()
    nc = _CACHE["nc"]

    x = np.asarray(x, np.float32)
    Wqkv = np.asarray(Wqkv, np.float32)
    bqkv = np.asarray(bqkv, np.float32)
    Wproj = np.asarray(Wproj, np.float32)
    bproj = np.asarray(bproj, np.float32)

    in_maps = _prep_inputs(x, Wqkv, bqkv, Wproj)
    res = run_bass_kernel_spmd(
        nc, in_maps, core_ids=list(range(N_CORES)), trace=_trace
    )
    if _trace_out is not None:
        _trace_out.append(res)

    out = np.empty((B, T, D), np.float32)
    for b in range(B):
        acc = res.results[HPC * b]["out"].astype(np.float32)
        for g in range(1, HPC):
            acc = acc + res.results[HPC * b + g]["out"]
        out[b] = acc + bproj[None, :]
    return out
